# revision 15
# baseline (speedup 1.0000x reference)
"""Multi-head attention (B=4, S=2048, d_model=1024, H=16) on 8 TRN2 NeuronCores.

Sharding: tensor-parallel over heads x data-parallel over batch.
Core c handles batch b=c//2 and head group g=c%2 (8 heads = 512 of the
1024 d_model columns of W_Q/W_K/W_V, and 512 rows of W_O). Each core
emits two fp16 partial outputs (W_O row-halves); the host sums the four
partials per batch.

v5 schedule (PE-paced; ~346us matmul stream vs ~268us exp on Scalar):
256 attention rounds of [scores pair (row-tiled K=64, concurrent) ->
exp -> attnV pair (LAG=5 behind)], with all projection work decomposed
into 2-matmul "atoms" drained at ~1/round from an EDF heap so the PE
never starves while exp paces. The output projection is split into
j01/j23 contraction halves writing separate fp16 partials: the j01
halves have no deadline and fill the r~208-240 zone after the dated
projection atoms run out; only the final block's j23 halves trail the
last exp. The last two blocks normalize via DVE reciprocal + a K=1
ones-matmul PE broadcast instead of the serial GpSimd chain. Input
DMAs are sliced fine and ordered by first-use so the first matmul
starts ~12us in. run_bass_kernel_spmd is retried (rare walrus
parallel-codegen flake).
"""
import numpy as np

B = 4
S = 2048
D = 1024
H = 16
DK = 64
NCORES = 8
HPC = 8          # heads per core
GCOLS = 512      # d_model columns per head group
QB = 512         # q-token block (PSUM bank free dim)
NQB = S // QB    # 4
NKB = S // 128   # 16 k-token blocks
NC_CHUNKS = D // 128  # 8 contraction chunks

_prog_cache = {}


def build_program(reps=1):
    """Build + compile the SPMD program."""
    key = (reps,)
    if key in _prog_cache:
        return _prog_cache[key]

    import heapq
    import itertools
    from collections import deque

    import concourse.bacc as bacc
    import concourse.mybir as mybir
    from concourse.tile import TileContext

    f16 = mybir.dt.float16
    f32 = mybir.dt.float32
    EXP = mybir.ActivationFunctionType.Exp

    nc = bacc.Bacc("TRN2", target_bir_lowering=False, debug=False,
                   num_devices=NCORES)

    # DRAM parameters (per-core shards, pre-laid-out on host).
    # X^T tensors are token-block-major so compute can start per-block.
    vt_d = nc.dram_tensor("vt", [NQB, 128, NC_CHUNKS, QB], f16,
                          kind="ExternalInput").ap()
    kt_d = nc.dram_tensor("kt", [NQB, 128, NC_CHUNKS, QB], f16,
                          kind="ExternalInput").ap()
    qt_d = nc.dram_tensor("qt", [NQB, 128, NC_CHUNKS, QB], f16,
                          kind="ExternalInput").ap()
    wq_d = nc.dram_tensor("wq", [128, NC_CHUNKS, GCOLS], f16, kind="ExternalInput").ap()
    wk_d = nc.dram_tensor("wk", [128, NC_CHUNKS, GCOLS], f16, kind="ExternalInput").ap()
    wv_d = nc.dram_tensor("wv", [128, NC_CHUNKS, GCOLS], f16, kind="ExternalInput").ap()
    wo_d = nc.dram_tensor("wo", [128, 4, D], f16, kind="ExternalInput").ap()
    yp_d = nc.dram_tensor("yp", [S, D], f16, kind="ExternalOutput").ap()
    yp2_d = nc.dram_tensor("yp2", [S, D], f16, kind="ExternalOutput").ap()

    with TileContext(nc) as tc:
        with tc.tile_pool(name="weights", bufs=1) as wpool, \
             tc.tile_pool(name="xt", bufs=1) as xtpool, \
             tc.tile_pool(name="kq", bufs=2) as kqpool, \
             tc.tile_pool(name="proj", bufs=1) as projpool, \
             tc.tile_pool(name="work", bufs=2) as workpool, \
             tc.tile_pool(name="psum", bufs=1, space="PSUM") as psp:

          for rep in range(reps):
            # ---- resident weights ----
            wq_sb = wpool.tile([128, NC_CHUNKS, GCOLS], f16, name="wq_sb", tag="wq")
            wk_sb = wpool.tile([128, NC_CHUNKS, GCOLS], f16, name="wk_sb", tag="wk")
            wv_sb = wpool.tile([128, NC_CHUNKS, GCOLS], f16, name="wv_sb", tag="wv")
            wo_sb = wpool.tile([128, 4, D], f16, name="wo_sb", tag="wo")
            ones64 = wpool.tile([1, 64], f16, name="ones64", tag="ones64")
            nc.vector.memset(ones64[:], 1.0)

            # ---- resident X^T inputs (all three live through the kernel:
            # deferred projection filler reads them late) ----
            vt_sb = xtpool.tile([128, NC_CHUNKS, S], f16, name="vt_sb", tag="vt")
            kt_sb = xtpool.tile([128, NC_CHUNKS, S], f16, name="kt_sb", tag="kt")
            qt_sb = xtpool.tile([128, NC_CHUNKS, S], f16, name="qt_sb", tag="qt")

            # ---- input DMAs, finest pieces first in consumption order ----
            def dma_slab(sb, dr, n):
                nc.sync.dma_start(out=sb[:, :, n * QB:(n + 1) * QB],
                                  in_=dr[n])

            def dma_tokpiece(sb, dr, n, p):
                # 128-token slice of slab n
                lo, hi = p * 128, (p + 1) * 128
                nc.sync.dma_start(
                    out=sb[:, :, n * QB + lo:n * QB + hi],
                    in_=dr[n][:, :, lo:hi])

            # critical path to the first matmuls, finely sliced
            nc.sync.dma_start(out=wv_sb[:, :, 0:256], in_=wv_d[:, :, 0:256])
            dma_tokpiece(vt_sb, vt_d, 0, 0)
            nc.sync.dma_start(out=wk_sb[:, :, 0:128], in_=wk_d[:, :, 0:128])
            nc.sync.dma_start(out=kt_sb[:, 0:4, 0:QB], in_=kt_d[0][:, 0:4, :])
            dma_tokpiece(vt_sb, vt_d, 0, 1)
            nc.sync.dma_start(out=kt_sb[:, 4:8, 0:QB], in_=kt_d[0][:, 4:8, :])
            nc.sync.dma_start(out=wq_sb[:, :, 0:128], in_=wq_d[:, :, 0:128])
            nc.sync.dma_start(out=qt_sb[:, 0:4, 0:QB], in_=qt_d[0][:, 0:4, :])
            dma_tokpiece(vt_sb, vt_d, 0, 2)
            nc.sync.dma_start(out=qt_sb[:, 4:8, 0:QB], in_=qt_d[0][:, 4:8, :])
            dma_tokpiece(vt_sb, vt_d, 0, 3)
            # remaining slabs in deadline order; weight remainders are
            # needed only from round ~63 so they follow the early slabs
            dma_slab(kt_sb, kt_d, 1)
            dma_slab(vt_sb, vt_d, 1)
            dma_slab(kt_sb, kt_d, 2)
            dma_slab(vt_sb, vt_d, 2)
            dma_slab(kt_sb, kt_d, 3)
            dma_slab(vt_sb, vt_d, 3)
            dma_slab(qt_sb, qt_d, 1)
            dma_slab(qt_sb, qt_d, 2)
            dma_slab(qt_sb, qt_d, 3)
            nc.sync.dma_start(out=wk_sb[:, :, 128:GCOLS],
                              in_=wk_d[:, :, 128:GCOLS])
            nc.sync.dma_start(out=wq_sb[:, :, 128:GCOLS],
                              in_=wq_d[:, :, 128:GCOLS])
            nc.sync.dma_start(out=wv_sb[:, :, 256:GCOLS],
                              in_=wv_d[:, :, 256:GCOLS])
            nc.sync.dma_start(out=wo_sb[:], in_=wo_d[:])

            # ---- projection outputs ----
            # kT/qT: [dk-on-partitions, token]; per-pair tiles (bufs=2):
            # head 2j on partitions 0:64, head 2j+1 on 64:128
            kT_t = {}
            qT_t = {}

            def kT(j):
                if j not in kT_t:
                    kT_t[j] = kqpool.tile([128, S], f16, name=f"kT{j}", tag="kT")
                return kT_t[j]

            def qT(j):
                if j not in qT_t:
                    qT_t[j] = kqpool.tile([128, S], f16, name=f"qT{j}", tag="qT")
                return qT_t[j]

            # v: [token-on-partitions, head, dim(+ones col at 64)]
            v_sb = projpool.tile([128, NKB, HPC, 66], f16, name="v_sb", tag="v")
            oT_sb = projpool.tile([128, 4, S], f16, name="oT_sb", tag="oT")
            for kb in range(NKB):
                nc.vector.memset(v_sb[:, kb, :, :], 1.0)

            # ---- PE work units, decomposed into 2-matmul atoms ----
            def v_atoms(kb, half):
                # project V for head pairs (2*half, 2*half+1) of block kb
                hold = [None]
                cols = slice(half * 256, half * 256 + 256)

                def mk(ci):
                    def f():
                        if ci == 0:
                            hold[0] = psp.tile([128, QB], f32, name="pps",
                                               tag="pps", bufs=2)
                        ps = hold[0]
                        for c in (2 * ci, 2 * ci + 1):
                            nc.tensor.matmul(
                                ps[:, 0:256],
                                vt_sb[:, c, kb * 128:(kb + 1) * 128],
                                wv_sb[:, c, cols],
                                start=(c == 0), stop=(c == NC_CHUNKS - 1))
                        if ci == 3:
                            nc.vector.tensor_copy(
                                v_sb[:, kb, 4 * half:4 * half + 4, 0:64],
                                ps[:, 0:256].rearrange("p (h d) -> p h d",
                                                       h=4))
                    return f
                return [mk(i) for i in range(4)]

            def proj_atoms(w_sb, xt, dst, j, n):
                # one K- or Q-projection tile block: 8 contraction chunks
                hold = [None]

                def mk(ci):
                    def f():
                        if ci == 0:
                            hold[0] = psp.tile([128, QB], f32, name="pps",
                                               tag="pps", bufs=2)
                        ps = hold[0]
                        for c in (2 * ci, 2 * ci + 1):
                            nc.tensor.matmul(
                                ps[:],
                                w_sb[:, c, j * 128:(j + 1) * 128],
                                xt[:, c, n * QB:(n + 1) * QB],
                                start=(c == 0), stop=(c == NC_CHUNKS - 1))
                        if ci == 3:
                            nc.vector.tensor_copy(
                                dst()[:, n * QB:(n + 1) * QB], ps[:])
                    return f
                return [mk(i) for i in range(4)]

            def outproj_atoms(t, n2, jhalf, tail=False):
                # half the contraction (j-groups 2*jhalf, 2*jhalf+1) into
                # its own partial output; the host sums the partials. This
                # halves the work gated on the final attention block and
                # gives the EDF queue deferrable PE work for the zone after
                # the last projection fillers run out.
                dst = yp_d if jhalf == 0 else yp2_d

                def f():
                    ps = psp.tile([128, QB], f32, name="pps",
                                  tag="pps", bufs=2)
                    for c2 in (2 * jhalf, 2 * jhalf + 1):
                        nc.tensor.matmul(
                            ps[:],
                            oT_sb[:, c2, t * 128:(t + 1) * 128],
                            wo_sb[:, c2, n2 * QB:(n2 + 1) * QB],
                            start=(c2 == 2 * jhalf),
                            stop=(c2 == 2 * jhalf + 1))
                    y_sb = workpool.tile([128, QB], f16, name="y_sb",
                                         tag="y", bufs=4)
                    if tail and (t + n2) % 2 == 0:
                        # post-attention: split drain copies across
                        # Scalar and DVE
                        nc.scalar.copy(y_sb[:], ps[:])
                    else:
                        nc.vector.tensor_copy(y_sb[:], ps[:])
                    nc.sync.dma_start(
                        out=dst[t * 128:(t + 1) * 128,
                                n2 * QB:(n2 + 1) * QB],
                        in_=y_sb[:])
                return [f]

            # ---- upfront PE work: the bare minimum before round 0 ----
            def run_chain(atoms_):
                for a in atoms_:
                    a()

            kT0a = proj_atoms(wk_sb, kt_sb, lambda: kT(0), 0, 0)
            qT0a = proj_atoms(wq_sb, qt_sb, lambda: qT(0), 0, 0)
            run_chain(v_atoms(0, 0))
            run_chain(kT0a)
            run_chain(v_atoms(1, 0))
            run_chain(qT0a)
            run_chain(v_atoms(2, 0))
            run_chain(v_atoms(3, 0))

            # ---- deferred atom queue: heap of (deadline, seq, fn) ----
            seq = itertools.count()
            pending = []

            def add_unit(dl, fns):
                for f in fns:
                    heapq.heappush(pending, (dl, next(seq), f))

            for kb in range(4, NKB):
                add_unit(kb, v_atoms(kb, 0))
            for n in range(1, NQB):
                add_unit(4 * n - 1,
                         proj_atoms(wk_sb, kt_sb, lambda: kT(0), 0, n))
            for n in range(1, NQB):
                add_unit(16 * n - 2,
                         proj_atoms(wq_sb, qt_sb, lambda: qT(0), 0, n))
            for j in range(1, 4):
                for n in range(NQB):
                    add_unit(64 * j - 1,
                             proj_atoms(wk_sb, kt_sb,
                                        lambda j=j: kT(j), j, n))
                for n in range(NQB):
                    add_unit(64 * j + 16 * n - 1,
                             proj_atoms(wq_sb, qt_sb,
                                        lambda j=j: qT(j), j, n))
            for kb in range(NKB):
                # true need: attnV for j-group 2 emitted at round 128+kb+LAG
                add_unit(130 + kb, v_atoms(kb, 1))

            def pop_atoms(k=1):
                while k > 0 and pending:
                    heapq.heappop(pending)[2]()
                    k -= 1

            # ---- flat attention pipeline ----
            # LAG deep enough that exp(r-LAG) has completed well before
            # attnV(r-LAG) is emitted, so the attnV pair is ready at its
            # priority slot and never throttles on the activation.
            LAG = 5
            rounds = [(j, qb, kb)
                      for j in range(4) for qb in range(NQB)
                      for kb in range(NKB)]
            pT_ring = {}
            outps = {}
            normq = deque()
            cur_r = [0]

            def queue_outproj(j, qb, r_now, tail):
                if j not in (1, 3):
                    return
                units = [(t, n2) for t in range(qb * 4, qb * 4 + 4)
                         for n2 in range(2)]
                if j == 1:
                    # deferrable: EDF pops these only once every dated
                    # atom is spent (the r~208-240 dry zone)
                    for t, n2 in units:
                        add_unit(10 ** 6, outproj_atoms(t, n2, 0))
                else:
                    for i, (t, n2) in enumerate(units):
                        add_unit(r_now + 1 + 2 * i,
                                 outproj_atoms(t, n2, 1, tail))

            def normalize(j, qb, unnorm0, unnorm1, rcp0, rcp1):
                def _run():
                    rcph = workpool.tile([1, 2, QB], f16, name="rcph",
                                         tag="rcph", bufs=1)
                    nc.gpsimd.tensor_copy(rcph[:, 0, :], rcp0[:])
                    nc.gpsimd.tensor_copy(rcph[:, 1, :], rcp1[:])
                    rbc = workpool.tile([64, 2, QB], f16, name="rbc",
                                        tag="rbc", bufs=1)
                    nc.gpsimd.partition_broadcast(rbc[:, 0, :],
                                                  rcph[0:1, 0, :])
                    nc.gpsimd.partition_broadcast(rbc[:, 1, :],
                                                  rcph[0:1, 1, :])
                    nc.vector.tensor_mul(
                        oT_sb[0:64, j, qb * QB:(qb + 1) * QB],
                        unnorm0[:], rbc[:, 0, :])
                    nc.vector.tensor_mul(
                        oT_sb[64:128, j, qb * QB:(qb + 1) * QB],
                        unnorm1[:], rbc[:, 1, :])
                    queue_outproj(j, qb, cur_r[0], tail=False)
                return _run

            def fast_normalize(j, qb, unnorm0, unnorm1, rcp0, rcp1, tail):
                # tail path: DVE cast + PE ones-matmul broadcast instead of
                # the ~4us serial GpSimd chain, so the last outproj units
                # are released while the PE is still warm
                rcp16 = workpool.tile([1, 2, QB], f16, name="rcp16",
                                      tag="rcph", bufs=1)
                nc.vector.tensor_copy(rcp16[:, 0, :], rcp0[:])
                nc.vector.tensor_copy(rcp16[:, 1, :], rcp1[:])
                bc = psp.tile([128, QB], f32, name="pps", tag="pps", bufs=2)
                nc.tensor.matmul(bc[0:64, :], ones64[:], rcp16[:, 0, :],
                                 start=True, stop=True)
                nc.tensor.matmul(bc[64:128, :], ones64[:], rcp16[:, 1, :],
                                 start=True, stop=True)
                nc.vector.tensor_mul(
                    oT_sb[0:64, j, qb * QB:(qb + 1) * QB],
                    unnorm0[:], bc[0:64, :])
                nc.vector.tensor_mul(
                    oT_sb[64:128, j, qb * QB:(qb + 1) * QB],
                    unnorm1[:], bc[64:128, :])
                queue_outproj(j, qb, cur_r[0], tail)

            for r in range(len(rounds) + LAG):
                cur_r[0] = r
                if r >= LAG:
                    jj, qq, kk = rounds[r - LAG]
                    if kk == 8 and normq:
                        # run the previous block's normalize mid-block: the
                        # gpsimd reciprocal-broadcast chain has long settled,
                        # so the DVE multiplies never stall the DVE queue
                        normq.popleft()()
                    if kk == 0:
                        # PSUM handoff: cover the staging latency
                        pop_atoms(2)
                        outps[(jj, qq)] = (
                            psp.tile([128, QB], f32, name="out0",
                                     tag="out0", bufs=1),
                            psp.tile([128, QB], f32, name="out1",
                                     tag="out1", bufs=1))
                    out0, out1 = outps[(jj, qq)]
                    pT = pT_ring.pop(r - LAG)
                    nc.tensor.matmul(
                        out0[0:65, :], v_sb[:, kk, 2 * jj, 0:65],
                        pT[:, 0, :],
                        start=(kk == 0), stop=(kk == NKB - 1))
                    nc.tensor.matmul(
                        out1[0:65, :], v_sb[:, kk, 2 * jj + 1, 0:65],
                        pT[:, 1, :],
                        start=(kk == 0), stop=(kk == NKB - 1))
                if r < len(rounds):
                    j, qb, kb = rounds[r]
                    # safety net: anything whose deadline is this round
                    # must be emitted before these scores read it
                    while pending and pending[0][0] <= r:
                        pop_atoms(1)
                    kTj, qTj = kT(j), qT(j)
                    sb2 = psp.tile([128, 2, QB], f32, name="sb2",
                                   tag="sbig", bufs=2)
                    # row-packed score pair: head 2j on PE rows 0:64,
                    # head 2j+1 on rows 64:128
                    nc.tensor.matmul(
                        sb2[:, 0, :],
                        kTj[0:64, kb * 128:(kb + 1) * 128],
                        qTj[0:64, qb * QB:(qb + 1) * QB],
                        start=True, stop=True)
                    nc.tensor.matmul(
                        sb2[:, 1, :],
                        kTj[64:128, kb * 128:(kb + 1) * 128],
                        qTj[64:128, qb * QB:(qb + 1) * QB],
                        start=True, stop=True)
                    pT = workpool.tile([128, 2, QB], f16, name="pT",
                                       tag="pT", bufs=LAG + 1)
                    nc.scalar.activation(
                        pT[:].rearrange("p a b -> p (a b)"),
                        sb2[:].rearrange("p a b -> p (a b)"),
                        EXP, scale=0.125)
                    pT_ring[r] = pT
                if r >= LAG:
                    jj, qq, kk = rounds[r - LAG]
                    if kk == NKB - 1:
                        # stage unnormalized output + denominators (SBUF),
                        # reciprocal off the staged row; frees both banks
                        unnorm0 = workpool.tile([64, QB], f16, name="un0",
                                                tag="un0", bufs=1)
                        unnorm1 = workpool.tile([64, QB], f16, name="un1",
                                                tag="un1", bufs=1)
                        db0 = workpool.tile([1, QB], f32, name="db0",
                                            tag="db0", bufs=1)
                        db1 = workpool.tile([1, QB], f32, name="db1",
                                            tag="db1", bufs=1)
                        rcp0 = workpool.tile([1, QB], f32, name="rcp0",
                                             tag="rcp0", bufs=1)
                        rcp1 = workpool.tile([1, QB], f32, name="rcp1",
                                             tag="rcp1", bufs=1)
                        nc.vector.tensor_copy(db0[:], out0[64:65, :])
                        nc.vector.tensor_copy(db1[:], out1[64:65, :])
                        nc.vector.reciprocal_approx_fast(rcp0[:], db0[:])
                        nc.vector.reciprocal_approx_fast(rcp1[:], db1[:])
                        if (jj, qq) == (3, 3):
                            # last block: exp is done, so stage on the idle
                            # Scalar engine in parallel with the DVE
                            # reciprocal chain
                            nc.scalar.copy(unnorm0[:], out0[0:64, :])
                            nc.scalar.copy(unnorm1[:], out1[0:64, :])
                            fast_normalize(jj, qq, unnorm0, unnorm1,
                                           rcp0, rcp1, tail=True)
                        else:
                            nc.vector.tensor_copy(unnorm0[:], out0[0:64, :])
                            nc.scalar.copy(unnorm1[:], out1[0:64, :])
                            if (jj, qq) == (3, 2):
                                fast_normalize(jj, qq, unnorm0, unnorm1,
                                               rcp0, rcp1, tail=False)
                            else:
                                normq.append(normalize(jj, qq, unnorm0,
                                                       unnorm1, rcp0, rcp1))
                        del outps[(jj, qq)]
                if pending:
                    pop_atoms(1)

            while normq:
                normq.popleft()()
            while pending:
                pop_atoms(1)

    nc.compile()
    _prog_cache[key] = nc
    return nc


def _chunk_pT(x):
    """[S, D] -> [128, D//128, S] fp16 (X^T chunked: out[p, c, t] = x[t, 128c+p])."""
    return np.ascontiguousarray(x.reshape(S, NC_CHUNKS, 128).transpose(2, 1, 0))


def _tok_blocks(xt, blk):
    """[128, NC, S] -> [S//blk, 128, NC, blk] token-block-major."""
    return np.ascontiguousarray(
        xt.reshape(128, NC_CHUNKS, S // blk, blk).transpose(2, 0, 1, 3))


def _chunk_w(w):
    """[D, GCOLS] -> [128, 8, GCOLS]: out[p, c, m] = w[128c+p, m]."""
    return np.ascontiguousarray(
        w.reshape(NC_CHUNKS, 128, w.shape[1]).transpose(1, 0, 2))


def prepare_in_maps(Q, K, V, W_Q, W_K, W_V, W_O):
    f16 = np.float16
    qt = [_tok_blocks(_chunk_pT(Q[b].astype(f16)), QB) for b in range(B)]
    kt = [_tok_blocks(_chunk_pT(K[b].astype(f16)), QB) for b in range(B)]
    vt = [_tok_blocks(_chunk_pT(V[b].astype(f16)), QB) for b in range(B)]
    wq = [_chunk_w(W_Q[:, g * GCOLS:(g + 1) * GCOLS].astype(f16)) for g in range(2)]
    wk = [_chunk_w(W_K[:, g * GCOLS:(g + 1) * GCOLS].astype(f16)) for g in range(2)]
    wv = [_chunk_w(W_V[:, g * GCOLS:(g + 1) * GCOLS].astype(f16)) for g in range(2)]
    # wo rows for group g, chunked: [128, 4, D]
    wo = [np.ascontiguousarray(
        W_O[g * GCOLS:(g + 1) * GCOLS, :].astype(f16)
        .reshape(4, 128, D).transpose(1, 0, 2)) for g in range(2)]
    in_maps = []
    for c in range(NCORES):
        b, g = c // 2, c % 2
        in_maps.append({
            "qt": qt[b], "kt": kt[b], "vt": vt[b],
            "wq": wq[g], "wk": wk[g], "wv": wv[g], "wo": wo[g],
        })
    return in_maps


def execute(nc, in_maps):
    from concourse.bass_utils import run_bass_kernel_spmd
    last = None
    for _ in range(3):
        try:
            return run_bass_kernel_spmd(nc, in_maps, list(range(NCORES)))
        except Exception as e:  # walrus parallel codegen flakes rarely
            last = e
    raise last


def _numpy_fallback(Q, K, V, mask, W_Q, W_K, W_V, W_O):
    import math
    B_, S1, _ = Q.shape
    q = (Q.reshape(-1, D) @ W_Q).reshape(B_, S1, H, DK).transpose(0, 2, 1, 3)
    k = (K.reshape(-1, D) @ W_K).reshape(B_, S1, H, DK).transpose(0, 2, 1, 3)
    v = (V.reshape(-1, D) @ W_V).reshape(B_, S1, H, DK).transpose(0, 2, 1, 3)
    out = np.empty((B_, H, S1, DK), np.float32)
    for b in range(B_):
        for h in range(H):
            s = (q[b, h] @ k[b, h].T) / math.sqrt(DK)
            s = np.where(mask[b] == 0, np.float32(-1e9), s)
            s = s - s.max(axis=-1, keepdims=True)
            e = np.exp(s)
            p = e / e.sum(axis=-1, keepdims=True)
            out[b, h] = p @ v[b, h]
    o = out.transpose(0, 2, 1, 3).reshape(B_, S1, D)
    return (o.reshape(-1, D) @ W_O).reshape(B_, S1, D).astype(np.float32)


def kernel(Q, K, V, mask, W_Q, W_K, W_V, W_O):
    Q = np.asarray(Q); K = np.asarray(K); V = np.asarray(V)
    mask = np.asarray(mask)
    W_Q = np.asarray(W_Q); W_K = np.asarray(W_K)
    W_V = np.asarray(W_V); W_O = np.asarray(W_O)
    if (mask == 0).any():
        # spec guarantees an all-ones mask; this path is correctness insurance
        return _numpy_fallback(Q, K, V, mask, W_Q, W_K, W_V, W_O)
    nc = build_program()
    in_maps = prepare_in_maps(Q, K, V, W_Q, W_K, W_V, W_O)
    res = execute(nc, in_maps)
    out = np.empty((B, S, D), np.float32)
    for b in range(B):
        r0, r1 = res.results[2 * b], res.results[2 * b + 1]
        out[b] = (r0["yp"] + r0["yp2"]) + (r1["yp"] + r1["yp2"])
    return out
